# revision 1
# baseline (speedup 1.0000x reference)
"""ECE loss (equal-width 15-bin) for [1048576, 128] logits on 8 TRN2 NeuronCores.

Strategy (data-parallel over rows, per the sharding hint):
  Device, per core (N/8 = 131072 rows):
    - stream [128 partitions, G rows, 128 classes] supertiles of y_pred
    - DVE:   grouped reduce_max over classes -> per-row max m (all rows)
    - row softmax denominators U = sum_c exp(x_c) (unshifted exp is safe:
      |x| <= ~6.5), split between two engines to balance their load:
        * rows [0, KA) of each supertile: one ACT activation per row with
          accum_out -> exp+sum fused on the Scalar engine
        * rows [KA, G): one batched ACT exp + one grouped DVE reduce_sum
    - outputs m, u_a, u_b -- a 512MB -> 1.5MB reduction
  Host:
    conf = exp(m)/U  (== max softmax);  acc = (y_pred[r, y_true[r]] == m)
    (the row max is an exact element of the row, so float equality
    reproduces argmax == label up to exact-tie rows), then the 15-bin
    equal-width histogram and the final ECE reduction as in the reference.

Measured: all-DVE reductions 311us (DVE busy 282us, ACT 119us, DMA floor
~195-205us/core). The KA split moves ~9/32 of row-sums to ACT's idle
capacity (each accum row costs ~557ns extra on ACT incl. the separate
ACTIVATION_READ_ACCUMULATOR), balancing both engines at ~250us busy;
with the geometric warm-up schedule, a KA=16 taper on the last four
supertiles (fills ACT's tail idle), and chunked output flushes, both
engines run gap-free and finish within ~1.5us of each other: ~263us
measured (slowest core; ~11us startup barriers/DMA latency + ~10us
drain/barrier tail are fixed costs).
"""

import numpy as np

import concourse.bacc as bacc
import concourse.tile as tile
from concourse import mybir
from concourse.bass_utils import run_bass_kernel_spmd

N_CORES = 8
N = 1048576
C = 128
N_SHARD = N // N_CORES  # 131072
P = 128                 # SBUF partitions
T = N_SHARD // P        # 1024 rows handled per partition
G = 32                  # rows per partition per (full) supertile
KA = 9                  # accum rows per 32 (exp+sum fused on ACT)
N_BINS = 15

# warm-up schedule: small leading supertiles so compute starts ~8us earlier.
# entries: (t0, g, ka); ua/ub columns are laid out in schedule order.
def _schedule():
    # geometric warm-up so DMA prefetch stays ahead of compute from the start;
    # the last supertiles carry extra accum rows (ACT otherwise idles ~14us
    # at the tail while DVE finishes its sums).
    gs = [8] * 8 + [16] * 4 + [32] * 28
    assert sum(gs) == T
    sched = []
    t0 = 0
    for i, g in enumerate(gs):
        ka = g * KA // 32
        if i >= len(gs) - 4 and g == 32:
            ka = 16
        sched.append((t0, g, ka))
        t0 += g
    return sched

SCHED = _schedule()
NA = sum(ka for _, _, ka in SCHED)          # total accum rows per partition
NB = sum(g - ka for _, g, ka in SCHED)      # total batched rows per partition

_CACHE: dict = {}


def _build_bass():
    nc = bacc.Bacc(None, target_bir_lowering=False)
    x = nc.dram_tensor("x", [N_SHARD, C], mybir.dt.float32, kind="ExternalInput")
    m_out = nc.dram_tensor("m_out", [N_SHARD], mybir.dt.float32, kind="ExternalOutput")
    ua_out = nc.dram_tensor("ua_out", [P * NA], mybir.dt.float32, kind="ExternalOutput")
    ub_out = nc.dram_tensor("ub_out", [P * NB], mybir.dt.float32, kind="ExternalOutput")

    # row r = p*T + t lives at [p, t]; per-partition runs in DRAM stay contiguous
    xv = x[:, :].rearrange("(p t) c -> p t c", p=P)
    mv = m_out[:].rearrange("(p t) -> p t", p=P)
    uav = ua_out[:].rearrange("(p t) -> p t", p=P)
    ubv = ub_out[:].rearrange("(p t) -> p t", p=P)

    with tile.TileContext(nc) as tc:
        with (
            tc.tile_pool(name="xin", bufs=8) as xin_pool,
            tc.tile_pool(name="exps", bufs=3) as exp_pool,
            tc.tile_pool(name="stats", bufs=1) as stats_pool,
        ):
            m_all = stats_pool.tile([P, T], mybir.dt.float32)
            ua_all = stats_pool.tile([P, NA], mybir.dt.float32)
            ub_all = stats_pool.tile([P, NB], mybir.dt.float32)
            a_off = 0
            b_off = 0
            m_flushed = 0
            a_flushed = 0
            b_flushed = 0
            for si, (t0, g, ka) in enumerate(SCHED):
                kb = g - ka
                xt = xin_pool.tile([P, g, C], mybir.dt.float32, tag="xt")
                nc.sync.dma_start(out=xt[:], in_=xv[:, t0 : t0 + g, :])
                nc.vector.reduce_max(
                    out=m_all[:, t0 : t0 + g],
                    in_=xt[:],
                    axis=mybir.AxisListType.X,
                )
                # ACT path: exp+sum fused, one instruction per row
                esc = exp_pool.tile([P, 1, C], mybir.dt.float32, tag="esc")
                for j in range(ka):
                    nc.scalar.activation(
                        out=esc[:],
                        in_=xt[:, j : j + 1, :],
                        func=mybir.ActivationFunctionType.Exp,
                        accum_out=ua_all[:, a_off + j : a_off + j + 1],
                    )
                # DVE path: batched exp then grouped reduce_sum
                et = exp_pool.tile([P, kb, C], mybir.dt.float32, tag="et")
                nc.scalar.activation(
                    out=et[:],
                    in_=xt[:, ka:g, :],
                    func=mybir.ActivationFunctionType.Exp,
                )
                nc.vector.reduce_sum(
                    out=ub_all[:, b_off : b_off + kb],
                    in_=et[:],
                    axis=mybir.AxisListType.X,
                )
                a_off += ka
                b_off += kb
                if si % 8 == 7 or si == len(SCHED) - 1:
                    nc.sync.dma_start(
                        out=mv[:, m_flushed : t0 + g], in_=m_all[:, m_flushed : t0 + g]
                    )
                    nc.sync.dma_start(
                        out=uav[:, a_flushed:a_off], in_=ua_all[:, a_flushed:a_off]
                    )
                    nc.sync.dma_start(
                        out=ubv[:, b_flushed:b_off], in_=ub_all[:, b_flushed:b_off]
                    )
                    m_flushed = t0 + g
                    a_flushed = a_off
                    b_flushed = b_off
    nc.finalize()
    return nc


def run_device(y_pred: np.ndarray, **spmd_kwargs):
    """Run the bass kernel on 8 cores; returns (m, U) each [N] f32 plus results obj."""
    if "nc" not in _CACHE:
        _CACHE["nc"] = _build_bass()
    nc = _CACHE["nc"]
    in_maps = [{"x": y_pred[c * N_SHARD : (c + 1) * N_SHARD]} for c in range(N_CORES)]
    res = run_bass_kernel_spmd(nc, in_maps, core_ids=list(range(N_CORES)), **spmd_kwargs)
    m = np.concatenate([r["m_out"] for r in res.results])
    # reassemble U: per core/partition, supertile rows [0,ka) came from the
    # ACT path (ua columns in schedule order), rows [ka,g) from DVE (ub)
    u_parts = []
    for r in res.results:
        ua = r["ua_out"].reshape(P, NA)
        ub = r["ub_out"].reshape(P, NB)
        u = np.empty((P, T), dtype=np.float32)
        a_off = b_off = 0
        for t0, g, ka in SCHED:
            u[:, t0 : t0 + ka] = ua[:, a_off : a_off + ka]
            u[:, t0 + ka : t0 + g] = ub[:, b_off : b_off + g - ka]
            a_off += ka
            b_off += g - ka
        u_parts.append(u.reshape(P * T))
    u = np.concatenate(u_parts)
    return m, u, res


def finish_host(y_pred, y_true, m, u) -> np.ndarray:
    xl = y_pred[np.arange(N), np.asarray(y_true, dtype=np.int64)]
    conf = np.exp(m.astype(np.float64)) / u.astype(np.float64)
    acc = (xl == m).astype(np.float64)
    bin_idx = np.clip(np.ceil(conf * N_BINS).astype(np.int64) - 1, 0, N_BINS - 1)
    cnt = np.bincount(bin_idx, minlength=N_BINS).astype(np.float64)
    conf_sum = np.bincount(bin_idx, weights=conf, minlength=N_BINS)
    acc_sum = np.bincount(bin_idx, weights=acc, minlength=N_BINS)
    safe = np.where(cnt > 0, cnt, 1.0)
    per_bin = np.where(cnt > 0, np.abs(conf_sum / safe - acc_sum / safe) * (cnt / N), 0.0)
    return np.array([per_bin.sum()], dtype=np.float32)


def kernel(y_pred: np.ndarray, y_true: np.ndarray) -> np.ndarray:
    y_pred = np.ascontiguousarray(np.asarray(y_pred, dtype=np.float32))
    m, u, _ = run_device(y_pred)
    return finish_host(y_pred, y_true, m, u)



# revision 3
# speedup vs baseline: 1.1588x; 1.1588x over previous
"""ECE loss (equal-width 15-bin) for [1048576, 128] logits on 8 TRN2 NeuronCores.

Strategy (data-parallel over rows, per the sharding hint):
  Device, per core (N/8 = 131072 rows):
    - stream [128 partitions, G rows, 128 classes] supertiles of y_pred
    - DVE: grouped f32 reduce_max over classes -> per-row max m (exact)
    - ACT: one batched exp per supertile, written as bf16
    - DVE: 4-level pairwise bf16 add tree (128 -> 8) at the 2x_1p DVE
      rate (0.5 cyc/elem; TensorTensor gets the 2-byte perf mode, while
      TensorReduce always runs 1 cyc/elem), then one grouped f32
      reduce_sum over the last 8 -> denominator U
    - outputs m, U -- a 512MB -> 1MB reduction
  Host:
    conf = exp(m)/U  (== max softmax);  acc = (y_pred[r, y_true[r]] == m)
    (the row max is an exact element of the row, so float equality
    reproduces argmax == label up to exact-tie rows), then the 15-bin
    equal-width histogram and the final ECE reduction as in the reference.

The bf16 sum tree perturbs U by ~8e-4 rms which moves the final ECE by
~2e-5 relative (simulated on the real inputs; gate is 2e-2). Engine
budget per core: ACT ~110us, DVE ~137(max) + ~64(tree) + ~9(tail)
= ~210us, vs a DMA input stream of 67.1MB (~165us on fast cores,
~230us on the slowest). The previous ACT/DVE-balanced split ran both
engines at ~250us busy -> 259-289us/core.
"""

import numpy as np

import concourse.bacc as bacc
import concourse.tile as tile
from concourse import mybir
from concourse.bass_utils import run_bass_kernel_spmd

N_CORES = 8
N = 1048576
C = 128
N_SHARD = N // N_CORES  # 131072
P = 128                 # SBUF partitions
T = N_SHARD // P        # 1024 rows handled per partition
N_BINS = 15
K_TREE = 4              # bf16 tree levels: 128 -> 8

# warm-up schedule: small leading supertiles so compute starts ~8us earlier
# and the DMA prefetch queue stays ahead of compute from the start.
def _schedule():
    gs = [8] * 8 + [16] * 4 + [32] * 28
    assert sum(gs) == T
    sched = []
    t0 = 0
    for g in gs:
        sched.append((t0, g))
        t0 += g
    return sched

SCHED = _schedule()

_CACHE: dict = {}


def _build_bass():
    nc = bacc.Bacc(None, target_bir_lowering=False)
    x = nc.dram_tensor("x", [N_SHARD, C], mybir.dt.float32, kind="ExternalInput")
    m_out = nc.dram_tensor("m_out", [N_SHARD], mybir.dt.float32, kind="ExternalOutput")
    u_out = nc.dram_tensor("u_out", [N_SHARD], mybir.dt.float32, kind="ExternalOutput")

    # row r = p*T + t lives at [p, t]; per-partition runs in DRAM stay contiguous
    xv = x[:, :].rearrange("(p t) c -> p t c", p=P)
    mv = m_out[:].rearrange("(p t) -> p t", p=P)
    uv = u_out[:].rearrange("(p t) -> p t", p=P)

    with tile.TileContext(nc) as tc:
        with (
            tc.tile_pool(name="xin", bufs=8) as xin_pool,
            tc.tile_pool(name="exps", bufs=3) as exp_pool,
            tc.tile_pool(name="tree", bufs=2) as tree_pool,
            tc.tile_pool(name="stats", bufs=1) as stats_pool,
            nc.allow_low_precision("bf16 pairwise sum tree; ECE impact ~2e-5 rel"),
        ):
            m_all = stats_pool.tile([P, T], mybir.dt.float32)
            u_all = stats_pool.tile([P, T], mybir.dt.float32)
            flushed = 0
            for si, (t0, g) in enumerate(SCHED):
                xt = xin_pool.tile([P, g, C], mybir.dt.float32, tag="xt")
                nc.sync.dma_start(out=xt[:], in_=xv[:, t0 : t0 + g, :])
                nc.vector.reduce_max(
                    out=m_all[:, t0 : t0 + g],
                    in_=xt[:],
                    axis=mybir.AxisListType.X,
                )
                et = exp_pool.tile([P, g, C], mybir.dt.bfloat16, tag="et")
                nc.scalar.activation(
                    out=et[:],
                    in_=xt[:],
                    func=mybir.ActivationFunctionType.Exp,
                )
                # bf16 pairwise tree 128 -> 8 at the 2-byte DVE rate
                src = et
                w = C
                for lvl in range(K_TREE):
                    w //= 2
                    dst = tree_pool.tile([P, g, w], mybir.dt.bfloat16, tag=f"s{lvl}")
                    nc.vector.tensor_tensor(
                        out=dst[:],
                        in0=src[:, :, 0:w],
                        in1=src[:, :, w : 2 * w],
                        op=mybir.AluOpType.add,
                    )
                    src = dst
                nc.vector.reduce_sum(
                    out=u_all[:, t0 : t0 + g],
                    in_=src[:],
                    axis=mybir.AxisListType.X,
                )
                if si % 8 == 7 or si == len(SCHED) - 1:
                    nc.sync.dma_start(
                        out=mv[:, flushed : t0 + g], in_=m_all[:, flushed : t0 + g]
                    )
                    nc.sync.dma_start(
                        out=uv[:, flushed : t0 + g], in_=u_all[:, flushed : t0 + g]
                    )
                    flushed = t0 + g
    nc.finalize()
    return nc


def run_device(y_pred: np.ndarray, **spmd_kwargs):
    """Run the bass kernel on 8 cores; returns (m, U) each [N] f32 plus results obj."""
    if "nc" not in _CACHE:
        _CACHE["nc"] = _build_bass()
    nc = _CACHE["nc"]
    in_maps = [{"x": y_pred[c * N_SHARD : (c + 1) * N_SHARD]} for c in range(N_CORES)]
    res = run_bass_kernel_spmd(nc, in_maps, core_ids=list(range(N_CORES)), **spmd_kwargs)
    m = np.concatenate([r["m_out"] for r in res.results])
    u = np.concatenate([r["u_out"] for r in res.results])
    return m, u, res


def finish_host(y_pred, y_true, m, u) -> np.ndarray:
    xl = y_pred[np.arange(N), np.asarray(y_true, dtype=np.int64)]
    conf = np.exp(m.astype(np.float64)) / u.astype(np.float64)
    acc = (xl == m).astype(np.float64)
    bin_idx = np.clip(np.ceil(conf * N_BINS).astype(np.int64) - 1, 0, N_BINS - 1)
    cnt = np.bincount(bin_idx, minlength=N_BINS).astype(np.float64)
    conf_sum = np.bincount(bin_idx, weights=conf, minlength=N_BINS)
    acc_sum = np.bincount(bin_idx, weights=acc, minlength=N_BINS)
    safe = np.where(cnt > 0, cnt, 1.0)
    per_bin = np.where(cnt > 0, np.abs(conf_sum / safe - acc_sum / safe) * (cnt / N), 0.0)
    return np.array([per_bin.sum()], dtype=np.float32)


def kernel(y_pred: np.ndarray, y_true: np.ndarray) -> np.ndarray:
    y_pred = np.ascontiguousarray(np.asarray(y_pred, dtype=np.float32))
    m, u, _ = run_device(y_pred)
    return finish_host(y_pred, y_true, m, u)
